# revision 26
# baseline (speedup 1.0000x reference)
"""BarrierNet forward Trainium2 kernel (8-core data parallel).

Strategy: pure data-parallel over batch (8 x 8192 samples); weights
replicated. Per core: the 3-layer MLP (64->256, 256->256 x2, heads
256->{2,1}) runs on the TensorEngine with transposed activations (batch on
the free dim) in fp16 with fp32 accumulation; relu evictions alternate
between ACT and DVE; the CVaR/GMM/QP epilogue runs per-sample in a
[128, 32]-plane layout, with the rel/vel-only subgraph precomputed during
the load phase and an erfinv that mirrors XLA's fp32 polynomials so errors
cancel against the jax reference.
"""

import os
import sys

for _p in ("/opt/trn_rl_repo", "/root/.axon_site/_ro/trn_rl_repo"):
    if os.path.isdir(_p) and _p not in sys.path:
        sys.path.insert(0, _p)

from contextlib import ExitStack

import numpy as np
import ml_dtypes

import concourse.bass as bass
import concourse.mybir as mybir
import bass_rust
from concourse import tile
from concourse.bass_utils import run_bass_kernel_spmd

F32 = mybir.dt.float32

# problem constants (hardcoded; kernel.py must be self-contained)
N_CORES = 8
B = 65536
BC = B // N_CORES          # 8192 samples per core
F = 64
H = 256
NT = 512                   # matmul moving free dim (one PSUM bank)
NTILES = BC // NT          # 16
SPL = NT + 128             # ACT/DVE eviction split point
HALF = BC // 2             # 4096
PC = 32                    # plane columns: [128, 32] per half

ALPHA = 2.0
BETA_MAX = 0.2
R_SAFE = 0.8
TAU = 0.1
GMM_STDS = (0.1, 0.3, 0.5)
INV_SQRT_2PI = float(1.0 / np.sqrt(2.0 * np.pi, dtype=np.float32))

# XLA fp32 ErfInv polynomial coefficients (Giles), highest degree first.
ERFINV_A = [2.81022636e-08, 3.43273939e-07, -3.5233877e-06, -4.39150654e-06,
            0.00021858087, -0.00125372503, -0.00417768164, 0.246640727,
            1.50140941]
ERFINV_B = [-0.000200214257, 0.000100950558, 0.00134934322, -0.00367342844,
            0.00573950773, -0.0076224613, 0.00943887047, 1.00167406,
            2.83297682]

MULT = mybir.AluOpType.mult
ADD = mybir.AluOpType.add
SUB = mybir.AluOpType.subtract
MAX = mybir.AluOpType.max
MIN = mybir.AluOpType.min
IS_GT = mybir.AluOpType.is_gt
AF = mybir.ActivationFunctionType

_MDT = {"bf16": mybir.dt.bfloat16, "fp16": mybir.dt.float16,
        "fp32r": F32, "fp32": F32}
_NPDT = {"bf16": ml_dtypes.bfloat16, "fp16": np.float16,
         "fp32r": np.float32, "fp32": np.float32}


def _split_multi_waits(nc):
    """This walrus build supports a single sync-wait per instruction; the
    TileContext tail drain can carry several.  Split the extras into
    standalone preceding Drain instructions on the same engine."""
    for fn in nc.m.functions:
        for bb in fn.blocks:
            new_insts = []
            changed = False
            for inst in bb.instructions:
                si = inst.sync_info
                waits = list(si.on_wait) if (si is not None and si.on_wait) else []
                if len(waits) > 1:
                    changed = True
                    for k, w in enumerate(waits[:-1]):
                        d = mybir.InstDrain(name=f"{inst.name}_sw{k}", ins=[], outs=[])
                        d.engine = inst.engine
                        d.sync_info = bass_rust.SyncInfo(on_wait=[w], on_update=[])
                        nc.register_instruction(d)
                        new_insts.append(d)
                    inst.sync_info = bass_rust.SyncInfo(
                        on_wait=[waits[-1]], on_update=list(si.on_update or []))
                new_insts.append(inst)
            if changed:
                bb.instructions = new_insts


def _build(mm="fp16", zb=True):
    """Build the per-core Bass program. mm in {'bf16','fp16','fp32r','fp32'}.
    zb: all MLP biases are zero -> merged [128,1024] evictions."""
    mdt = _MDT[mm]
    if mm == "fp32r":
        mc = lambda ap: ap.bitcast(mybir.dt.float32r)
    else:
        mc = lambda ap: ap

    nc = bass.Bass("TRN2", target_bir_lowering=False, debug=False,
                   num_devices=N_CORES)

    # packed obs: rows 0:64 = features of samples 0..4095, rows 64:128 =
    # features of samples 4096..8191 (full-width DMA); w1 duplicated rows.
    obs_t = nc.dram_tensor("obs_t", [128, BC // 2], mdt, kind="ExternalInput").ap()
    relvel = nc.dram_tensor("relvel", [128, 8 * PC], F32, kind="ExternalInput").ap()
    w1t = nc.dram_tensor("w1t", [128, H], mdt, kind="ExternalInput").ap()
    w21t = nc.dram_tensor("w21t", [H, H], mdt, kind="ExternalInput").ap()
    w22t = nc.dram_tensor("w22t", [H, H], mdt, kind="ExternalInput").ap()
    wht = nc.dram_tensor("wht", [2 * H, 3], mdt, kind="ExternalInput").ap()
    b1p = nc.dram_tensor("b1p", [128, 2], F32, kind="ExternalInput").ap()
    b21p = nc.dram_tensor("b21p", [128, 2], F32, kind="ExternalInput").ap()
    b22p = nc.dram_tensor("b22p", [128, 2], F32, kind="ExternalInput").ap()
    bhp = nc.dram_tensor("bhp", [128, 1], F32, kind="ExternalInput").ap()
    u_out = nc.dram_tensor("u", [BC, 2], F32, kind="ExternalOutput").ap()

    with tile.TileContext(nc) as tc, ExitStack() as ctx:
        const = ctx.enter_context(tc.tile_pool(name="const", bufs=1))
        mpsum = ctx.enter_context(tc.tile_pool(name="mpsum", bufs=3, space="PSUM"))
        hpsum = ctx.enter_context(tc.tile_pool(name="hpsum", bufs=2, space="PSUM"))
        xpool = ctx.enter_context(tc.tile_pool(name="xp", bufs=3))
        ypool = ctx.enter_context(tc.tile_pool(name="yp", bufs=4))
        hsbp = ctx.enter_context(tc.tile_pool(name="hsb", bufs=2))
        ep = ctx.enter_context(tc.tile_pool(name="ep", bufs=2))
        plp = ctx.enter_context(tc.tile_pool(name="pl", bufs=2))

        # loads: latency-critical tensors on the SP HWDGE queue in first-
        # needed order; the rest via gpsimd SWDGE so the ACT engine is free
        # to start evictions immediately.
        w1_sb = const.tile([128, H], mdt, tag="w1", name="w1_sb")
        nc.sync.dma_start(out=w1_sb[:], in_=w1t[:])
        obs_sb = const.tile([128, BC // 2], mdt, tag="obs", name="obs_sb")
        nc.sync.dma_start(out=obs_sb[:, 0:1024], in_=obs_t[:, 0:1024])
        w21_sb = [const.tile([128, H], mdt, tag=f"w21_{k}", name=f"w21_sb{k}")
                  for k in range(2)]
        w22_sb = [const.tile([128, H], mdt, tag=f"w22_{k}", name=f"w22_sb{k}")
                  for k in range(2)]
        for k in range(2):
            nc.sync.dma_start(out=w21_sb[k][:], in_=w21t[k * 128:(k + 1) * 128, :])
        for q in range(1, 4):
            nc.sync.dma_start(out=obs_sb[:, q * 1024:(q + 1) * 1024],
                              in_=obs_t[:, q * 1024:(q + 1) * 1024])
        for k in range(2):
            nc.gpsimd.dma_start(out=w22_sb[k][:], in_=w22t[k * 128:(k + 1) * 128, :])
        wh_sb = [const.tile([128, 3], mdt, tag=f"wh_{k}", name=f"wh_sb{k}")
                 for k in range(4)]
        for k in range(4):
            nc.gpsimd.dma_start(out=wh_sb[k][:], in_=wht[k * 128:(k + 1) * 128, :])
        bh_sb = const.tile([128, 1], F32, tag="bh", name="bh_sb")
        nc.gpsimd.dma_start(out=bh_sb[:], in_=bhp[:])
        rv_sb = const.tile([128, 8 * PC], F32, tag="rv", name="rv_sb")
        nc.gpsimd.dma_start(out=rv_sb[:], in_=relvel[:])
        b1_sb = const.tile([128, 2], F32, tag="b1", name="b1_sb")
        b21_sb = const.tile([128, 2], F32, tag="b21", name="b21_sb")
        b22_sb = const.tile([128, 2], F32, tag="b22", name="b22_sb")
        if not zb:
            nc.gpsimd.dma_start(out=b1_sb[:], in_=b1p[:])
            nc.gpsimd.dma_start(out=b21_sb[:], in_=b21p[:])
            nc.gpsimd.dma_start(out=b22_sb[:], in_=b22p[:])
        eps8 = const.tile([128, 1], F32, tag="eps8", name="eps8")
        nc.vector.memset(eps8[:], 1e-8)

        # ---- PE warm-up + ACT table pin, hidden under the input DMAs ----
        scratch = const.tile([128, NT], mdt, tag="scr", name="scratch")
        nc.vector.memset(scratch[:], 0.0)
        scr2 = const.tile([128, 8], F32, tag="scr2", name="scratch2")
        nc.vector.memset(scr2[:], 1.0)
        nc.scalar.activation(scr2[:], scr2[:], AF.Ln)
        nc.scalar.activation(scr2[:], scr2[:], AF.Exp)
        wps = mpsum.tile([128, 2 * NT], F32, tag="ps", name="warm_ps")
        for _ in range(8):
            nc.tensor.matmul(wps[:, 0:NT], lhsT=mc(scratch[:, 0:128]),
                             rhs=mc(scratch[:]), start=True, stop=True)

        # ---- eviction engine alternation ----
        ev_state = [0]
        EV_PATTERN = "ADADA"  # 3 ACT : 2 DVE

        def evict(dst, src, bias_ap, relu=True):
            e = EV_PATTERN[ev_state[0] % len(EV_PATTERN)]
            ev_state[0] += 1
            if e == "A":
                if bias_ap is None:
                    nc.scalar.activation(dst, src, AF.Relu if relu else AF.Copy)
                else:
                    nc.scalar.activation(dst, src,
                                         AF.Relu if relu else AF.Identity,
                                         bias=bias_ap)
            else:
                if bias_ap is None:
                    if relu:
                        nc.vector.tensor_scalar(dst, src, 0.0, None, MAX)
                    else:
                        nc.vector.tensor_copy(dst, src)
                elif relu:
                    nc.vector.tensor_scalar(dst, src, bias_ap, 0.0, ADD, MAX)
                else:
                    nc.vector.tensor_scalar(dst, src, bias_ap, None, ADD)

        # ---- per-half head-output planes (written by spread DMAs) ----
        planes = []
        for h in range(2):
            planes.append({nm: plp.tile([128, PC], F32, tag=f"pl_{nm}",
                                        name=f"pl_{nm}_{h}")
                           for nm in ("ux", "uy", "bl")})

        V, S, G = nc.vector, nc.scalar, nc.gpsimd

        def T(tag):
            return ep.tile([128, PC], F32, tag=tag, name=f"ep_{tag}")[:]

        # ---- rel/vel-only subgraph: precomputed early (hidden in load) ----
        pre = [{}, {}]

        def pre_epilogue(h):
            rx = rv_sb[:, (0 + h) * PC:(1 + h) * PC]
            ry = rv_sb[:, (2 + h) * PC:(3 + h) * PC]
            vx = rv_sb[:, (4 + h) * PC:(5 + h) * PC]
            vy = rv_sb[:, (6 + h) * PC:(7 + h) * PC]
            t1 = T("t1"); V.tensor_tensor(t1, rx, rx, MULT)
            t2 = T("t2"); V.tensor_tensor(t2, ry, ry, MULT)
            rel2 = T("rel2"); V.tensor_tensor(rel2, t1, t2, ADD)
            d1 = T("d1"); G.tensor_tensor(d1, vx, rx, MULT)
            d2 = T("d2"); G.tensor_tensor(d2, vy, ry, MULT)
            ds = T("ds"); G.tensor_tensor(ds, d1, d2, ADD)
            X = T("X"); V.tensor_scalar(X, ds, 2.0, ALPHA * R_SAFE * R_SAFE,
                                        MULT, ADD)
            base = T("base"); V.scalar_tensor_tensor(base, rel2, -ALPHA, X,
                                                     MULT, ADD)
            sgm = []
            for mi, sd in enumerate(GMM_STDS):
                lr = T(f"lr{mi}")
                S.activation(lr, rel2, AF.Ln, bias=eps8[:], scale=4.0 * sd * sd)
                sf = T(f"sf{mi}")
                # sigma_f*INV_SQRT_2PI = exp(0.5*ln(v) + ln(INV_SQRT_2PI))
                S.activation(sf, lr, AF.Exp, scale=0.5)
                sg = T(f"sg{mi}")
                V.tensor_scalar(sg, sf, INV_SQRT_2PI, None, MULT)
                sgm.append(sg)
            den = T("den"); V.tensor_scalar(den, rel2, 4.0, 1e-12, MULT, ADD)
            rden = T("rden"); V.reciprocal(rden, den)
            pre[h] = dict(rx=rx, ry=ry, base=base, sgm=sgm, rden=rden)

        pre_epilogue(0)
        pre_epilogue(1)

        # ---- per-half epilogue chain (heads-dependent part) ----
        def epilogue(h):
            C = G if h == 0 else V  # hidden half -> gpsimd, exposed -> DVE
            p = pre[h]
            rx, ry, base, sgm, rden = p["rx"], p["ry"], p["base"], p["sgm"], p["rden"]
            ux = planes[h]["ux"][:]
            uy = planes[h]["uy"][:]
            bl = planes[h]["bl"][:]

            esg = T("esg"); S.activation(esg, bl, AF.Exp, scale=-1.0)
            rbeta = T("rbeta")
            V.tensor_scalar(rbeta, esg, 1.0 / BETA_MAX, 1.0 / BETA_MAX, MULT, ADD)
            V.tensor_scalar(rbeta, rbeta, 1e6, None, MIN)
            se = T("se"); V.tensor_scalar(se, esg, 1.0, None, ADD)
            rse = T("rse"); V.reciprocal(rse, se)
            beta = T("beta"); V.tensor_scalar(beta, rse, BETA_MAX, 1e-6, MULT, MAX)
            xb = T("xb"); V.tensor_scalar(xb, beta, -2.0, 1.0, MULT, ADD)
            tb = T("tb"); C.tensor_tensor(tb, xb, xb, MULT)
            wn = T("wn"); S.activation(wn, tb, AF.Ln, bias=1.0, scale=-1.0)
            wa = T("wa"); V.tensor_scalar(wa, wn, -1.0, -2.5, MULT, ADD)
            lw = T("lw"); S.activation(lw, wn, AF.Ln, scale=-1.0)
            sqw = T("sqw"); S.activation(sqw, lw, AF.Exp, scale=0.5)
            wb = T("wb"); V.tensor_scalar(wb, sqw, 3.0, None, SUB)

            # lateral: u-dependent dot product (off the beta chain)
            a1 = T("a1"); G.tensor_tensor(a1, rx, ux, MULT)
            a2 = T("a2"); G.tensor_tensor(a2, ry, uy, MULT)
            asum = T("as"); G.tensor_tensor(asum, a1, a2, ADD)

            def poly(w, C, tag, eng):
                pa = T(tag + "0"); pb_ = T(tag + "1")
                eng.tensor_scalar(pa, w, float(C[0]), None, MULT)
                cur, nxt = pa, pb_
                for c in C[1:8]:
                    eng.scalar_tensor_tensor(nxt, cur, float(c), w, ADD, MULT)
                    cur, nxt = nxt, cur
                out = T(tag + "r")
                eng.tensor_scalar(out, cur, float(C[8]), None, ADD)
                return out

            pa = poly(wa, ERFINV_A, "pA", V)
            pb = poly(wb, ERFINV_B, "pB", V)
            msk = ep.tile([128, PC], mybir.dt.uint8, tag="msk", name="ep_msk")[:]
            V.tensor_scalar(msk, wn, -5.0, None, IS_GT)
            psel = T("psel"); V.select(psel, msk, pa, pb)

            tt = T("tt"); C.tensor_tensor(tt, psel, xb, MULT)
            t2e = T("t2e"); C.tensor_tensor(t2e, tt, tt, MULT)
            pdfe = T("pdfe"); S.activation(pdfe, t2e, AF.Exp, scale=-1.0)
            cfp = T("cfp"); C.tensor_tensor(cfp, pdfe, rbeta, MULT)

            rhs = []
            for mi in range(3):
                tm = T(f"tm{mi}"); C.tensor_tensor(tm, sgm[mi], cfp, MULT)
                rr = T(f"rr{mi}"); C.tensor_tensor(rr, tm, base, ADD)
                rhs.append(rr)
            m1 = T("m1"); V.tensor_tensor(m1, rhs[0], rhs[1], MAX)
            mm_ = T("mm"); V.tensor_tensor(mm_, m1, rhs[2], MAX)
            es = []
            for mi in range(3):
                dd = T(f"dd{mi}"); V.tensor_tensor(dd, rhs[mi], mm_, SUB)
                ee = T(f"ee{mi}"); S.activation(ee, dd, AF.Exp, scale=1.0 / TAU)
                es.append(ee)
            s1 = T("s1"); C.tensor_tensor(s1, es[0], es[1], ADD)
            s2 = T("s2"); C.tensor_tensor(s2, s1, es[2], ADD)
            lse = T("lse"); S.activation(lse, s2, AF.Ln)
            rwc = T("rwc"); V.scalar_tensor_tensor(rwc, lse, TAU, mm_, MULT, ADD)

            viol = T("vi"); V.scalar_tensor_tensor(viol, asum, -2.0, rwc, MULT, ADD)
            vr = T("vr"); V.tensor_scalar(vr, viol, 0.0, None, MAX)
            stp = T("stp"); C.tensor_tensor(stp, vr, rden, MULT)
            sx = T("sx"); C.tensor_tensor(sx, stp, rx, MULT)
            sy = T("sy"); G.tensor_tensor(sy, stp, ry, MULT)
            uxy = plp.tile([128, 2 * PC], F32, tag="uxy", name="uxy")
            uxv = uxy.rearrange("p (c j) -> p c j", j=2)
            V.scalar_tensor_tensor(uxv[:, :, 0], sx, 2.0, ux, MULT, ADD)
            V.scalar_tensor_tensor(uxv[:, :, 1], sy, 2.0, uy, MULT, ADD)

            dst = u_out[h * HALF:(h + 1) * HALF, :].rearrange(
                "(p c) j -> p (c j)", p=128)
            nc.sync.dma_start(out=dst, in_=uxy[:])

        # ---- main batch loop ----
        # merged PSUM layout: one [128, 1024] tile holds both m-chunks;
        # heads for tile n-1 are emitted during iteration n so the PE never
        # stalls on the y-eviction it needs.
        pend = []

        def emit_heads_pair(items):
            (n0, yt0), (n1, yt1) = items
            hq = hpsum.tile([64, NT], F32, tag="hp", name="hp")

            def rhs_of(yt_, k):
                nm = "y21" if k < 2 else "y22"
                kk = k % 2
                return mc(yt_[nm][:, kk * NT:(kk + 1) * NT])

            for k in range(4):
                for off, yt_ in ((0, yt0), (32, yt1)):
                    nc.tensor.matmul(hq[off:off + 3, :], lhsT=mc(wh_sb[k][:]),
                                     rhs=rhs_of(yt_, k),
                                     start=(k == 0), stop=(k == 3))
            p2 = n1 // 2
            h_ = n1 // (NTILES // 2)
            hs = hsbp.tile([64, NT], F32, tag="hs", name="hs")
            evict(hs[:], hq[:], bh_sb[0:64, 0:1], relu=False)
            r0 = (p2 % 4) * 32
            for r, nm in enumerate(("ux", "uy", "bl")):
                nc.sync.dma_start(out=planes[h_][nm][r0:r0 + 32, :],
                                  in_=hs[r:r + 33:32, :])
            if n1 == NTILES // 2 - 1:
                epilogue(0)

        for n in range(NTILES):
            rows = slice(0, 64) if n < 8 else slice(64, 128)
            cc = (n % 8) * NT
            cols = slice(cc, cc + NT)
            # L1
            ps1 = mpsum.tile([128, 2 * NT], F32, tag="ps", name="ps1")
            for m in range(2):
                nc.tensor.matmul(ps1[:, m * NT:(m + 1) * NT],
                                 lhsT=mc(w1_sb[rows, m * 128:(m + 1) * 128]),
                                 rhs=mc(obs_sb[rows, cols]), start=True, stop=True)
            xt = xpool.tile([128, 2 * NT], mdt, tag="x", name="xt")
            if zb:
                nc.scalar.activation(xt[:, 0:SPL], ps1[:, 0:SPL], AF.Relu)
                nc.vector.tensor_scalar(xt[:, SPL:2 * NT], ps1[:, SPL:2 * NT],
                                        0.0, None, MAX)
            else:
                for m in range(2):
                    evict(xt[:, m * NT:(m + 1) * NT],
                          ps1[:, m * NT:(m + 1) * NT], b1_sb[:, m:m + 1])
            # L21 / L22
            yt = {}
            for wsb, bsb, nm in ((w21_sb, b21_sb, "y21"),
                                 (w22_sb, b22_sb, "y22")):
                ps = mpsum.tile([128, 2 * NT], F32, tag="ps", name="ps2")
                for k in range(2):
                    for m in range(2):
                        nc.tensor.matmul(
                            ps[:, m * NT:(m + 1) * NT],
                            lhsT=mc(wsb[k][:, m * 128:(m + 1) * 128]),
                            rhs=mc(xt[:, k * NT:(k + 1) * NT]),
                            start=(k == 0), stop=(k == 1))
                t = ypool.tile([128, 2 * NT], mdt, tag=nm, name=nm)
                if zb:
                    nc.scalar.activation(t[:, 0:SPL], ps[:, 0:SPL], AF.Relu)
                    nc.vector.tensor_scalar(t[:, SPL:2 * NT], ps[:, SPL:2 * NT],
                                            0.0, None, MAX)
                else:
                    for m in range(2):
                        evict(t[:, m * NT:(m + 1) * NT],
                              ps[:, m * NT:(m + 1) * NT], bsb[:, m:m + 1])
                yt[nm] = t
            if len(pend) == 2:
                emit_heads_pair(pend)
                pend = []
            pend.append((n, yt))
        emit_heads_pair(pend)
        epilogue(1)

    _split_multi_waits(nc)
    return nc


_CACHE = {}


def _get_nc(mm, zb):
    key = (mm, zb)
    if key not in _CACHE:
        _CACHE[key] = _build(mm, zb)
    return _CACHE[key]


def _prep_inputs(inputs, mm):
    obs = np.ascontiguousarray(inputs["obs"], dtype=np.float32)
    W1 = np.asarray(inputs["W1"], np.float32)
    b1 = np.asarray(inputs["b1"], np.float32)
    W21 = np.asarray(inputs["W21"], np.float32)
    b21 = np.asarray(inputs["b21"], np.float32)
    W22 = np.asarray(inputs["W22"], np.float32)
    b22 = np.asarray(inputs["b22"], np.float32)
    W31 = np.asarray(inputs["W31"], np.float32)
    b31 = np.asarray(inputs["b31"], np.float32)
    W32 = np.asarray(inputs["W32"], np.float32)
    b32 = np.asarray(inputs["b32"], np.float32)

    npdt = _NPDT[mm]

    w1t = np.ascontiguousarray(np.vstack([W1.T, W1.T])).astype(npdt)
    w21t = np.ascontiguousarray(W21.T).astype(npdt)
    w22t = np.ascontiguousarray(W22.T).astype(npdt)
    wht = np.zeros((2 * H, 3), np.float32)
    wht[0:H, 0:2] = W31.T
    wht[H:2 * H, 2] = W32[0]
    wht = wht.astype(npdt)
    b1p = np.ascontiguousarray(b1.reshape(2, 128).T)
    b21p = np.ascontiguousarray(b21.reshape(2, 128).T)
    b22p = np.ascontiguousarray(b22.reshape(2, 128).T)
    bhp = np.zeros((128, 1), np.float32)   # pair-packed head bias pattern
    for j in range(4):
        bhp[32 * j + 0, 0] = b31[0]
        bhp[32 * j + 1, 0] = b31[1]
        bhp[32 * j + 2, 0] = b32[0]
    zb = not (b1.any() or b21.any() or b22.any())

    in_maps = []
    for c in range(N_CORES):
        oc = obs[c * BC:(c + 1) * BC]                          # [8192, 64]
        obs_tc = np.empty((128, BC // 2), np.float32)
        obs_tc[0:64] = oc[0:BC // 2].T
        obs_tc[64:128] = oc[BC // 2:].T
        obs_tc = np.ascontiguousarray(obs_tc).astype(npdt)     # [128, 4096]
        rv = np.empty((128, 8 * PC), np.float32)
        for v, col in enumerate((6, 7, 8, 9)):
            pl = oc[:, col].reshape(2, 128, PC)                # [h][128, 32]
            rv[:, (2 * v) * PC:(2 * v + 1) * PC] = pl[0]
            rv[:, (2 * v + 1) * PC:(2 * v + 2) * PC] = pl[1]
        in_maps.append({
            "obs_t": obs_tc, "relvel": rv,
            "w1t": w1t, "w21t": w21t, "w22t": w22t, "wht": wht,
            "b1p": b1p, "b21p": b21p, "b22p": b22p, "bhp": bhp,
        })
    return in_maps, zb


def _install_ntff_hook_shim():
    """antenv.axon_hooks is absent in this image; recreate it from the
    boot helper so run_bass_kernel_spmd(trace=True) can NTFF-profile."""
    import types
    try:
        import antenv.axon_hooks  # noqa: F401
        return
    except ImportError:
        pass
    import antenv
    from trn_agent_boot.trn_boot import _ntff_profile_via_ctypes
    mod = types.ModuleType("antenv.axon_hooks")
    state = {"hook": _ntff_profile_via_ctypes("/opt/axon/libaxon_pjrt.so")}
    mod.set_axon_ntff_profile_hook = lambda h: state.__setitem__("hook", h)
    mod.get_axon_ntff_profile_hook = lambda: state["hook"]
    sys.modules["antenv.axon_hooks"] = mod
    antenv.axon_hooks = mod


def run(inputs, mm=None, trace=False, trace_kwargs=None):
    mm = mm or os.environ.get("BARRIER_MM_DTYPE", "fp16")
    if trace:
        _install_ntff_hook_shim()
    in_maps, zb = _prep_inputs(inputs, mm)
    nc = _get_nc(mm, zb)
    res = run_bass_kernel_spmd(nc, in_maps, list(range(N_CORES)),
                               trace=trace, **(trace_kwargs or {}))
    out = np.concatenate([res.results[c]["u"] for c in range(N_CORES)], axis=0)
    return out.astype(np.float32), res


def kernel(**inputs):
    out, _ = run(inputs)
    return out


# revision 27
# speedup vs baseline: 1.1765x; 1.1765x over previous
"""BarrierNet forward Trainium2 kernel (8-core data parallel).

Strategy: pure data-parallel over batch (8 x 8192 samples); weights
replicated. Per core: the 3-layer MLP (64->256, 256->256 x2, heads
256->{2,1}) runs on the TensorEngine with transposed activations (batch on
the free dim) in fp16 with fp32 accumulation; relu evictions alternate
between ACT and DVE; the CVaR/GMM/QP epilogue runs per-sample in a
[128, 32]-plane layout, with the rel/vel-only subgraph precomputed during
the load phase and an erfinv that mirrors XLA's fp32 polynomials so errors
cancel against the jax reference.
"""

import os
import sys

for _p in ("/opt/trn_rl_repo", "/root/.axon_site/_ro/trn_rl_repo"):
    if os.path.isdir(_p) and _p not in sys.path:
        sys.path.insert(0, _p)

from contextlib import ExitStack

import numpy as np
import ml_dtypes

import concourse.bass as bass
import concourse.mybir as mybir
import bass_rust
from concourse import tile
from concourse.bass_utils import run_bass_kernel_spmd

F32 = mybir.dt.float32

# problem constants (hardcoded; kernel.py must be self-contained)
N_CORES = 8
B = 65536
BC = B // N_CORES          # 8192 samples per core
F = 64
H = 256
NT = 512                   # matmul moving free dim (one PSUM bank)
NTILES = BC // NT          # 16
SPL = NT + 128             # ACT/DVE eviction split point
HALF = BC // 2             # 4096
PC = 32                    # plane columns: [128, 32] per half

ALPHA = 2.0
BETA_MAX = 0.2
R_SAFE = 0.8
TAU = 0.1
GMM_STDS = (0.1, 0.3, 0.5)
INV_SQRT_2PI = float(1.0 / np.sqrt(2.0 * np.pi, dtype=np.float32))

# XLA fp32 ErfInv polynomial coefficients (Giles), highest degree first.
ERFINV_A = [2.81022636e-08, 3.43273939e-07, -3.5233877e-06, -4.39150654e-06,
            0.00021858087, -0.00125372503, -0.00417768164, 0.246640727,
            1.50140941]
ERFINV_B = [-0.000200214257, 0.000100950558, 0.00134934322, -0.00367342844,
            0.00573950773, -0.0076224613, 0.00943887047, 1.00167406,
            2.83297682]

MULT = mybir.AluOpType.mult
ADD = mybir.AluOpType.add
SUB = mybir.AluOpType.subtract
MAX = mybir.AluOpType.max
MIN = mybir.AluOpType.min
IS_GT = mybir.AluOpType.is_gt
AF = mybir.ActivationFunctionType

_MDT = {"bf16": mybir.dt.bfloat16, "fp16": mybir.dt.float16,
        "fp32r": F32, "fp32": F32}
_NPDT = {"bf16": ml_dtypes.bfloat16, "fp16": np.float16,
         "fp32r": np.float32, "fp32": np.float32}


def _split_multi_waits(nc):
    """This walrus build supports a single sync-wait per instruction; the
    TileContext tail drain can carry several.  Split the extras into
    standalone preceding Drain instructions on the same engine."""
    for fn in nc.m.functions:
        for bb in fn.blocks:
            new_insts = []
            changed = False
            for inst in bb.instructions:
                si = inst.sync_info
                waits = list(si.on_wait) if (si is not None and si.on_wait) else []
                if len(waits) > 1:
                    changed = True
                    for k, w in enumerate(waits[:-1]):
                        d = mybir.InstDrain(name=f"{inst.name}_sw{k}", ins=[], outs=[])
                        d.engine = inst.engine
                        d.sync_info = bass_rust.SyncInfo(on_wait=[w], on_update=[])
                        nc.register_instruction(d)
                        new_insts.append(d)
                    inst.sync_info = bass_rust.SyncInfo(
                        on_wait=[waits[-1]], on_update=list(si.on_update or []))
                new_insts.append(inst)
            if changed:
                bb.instructions = new_insts


def _build(mm="fp16", zb=True):
    """Build the per-core Bass program. mm in {'bf16','fp16','fp32r','fp32'}.
    zb: all MLP biases are zero -> merged [128,1024] evictions."""
    mdt = _MDT[mm]
    if mm == "fp32r":
        mc = lambda ap: ap.bitcast(mybir.dt.float32r)
    else:
        mc = lambda ap: ap

    nc = bass.Bass("TRN2", target_bir_lowering=False, debug=False,
                   num_devices=N_CORES)

    # packed obs: rows 0:64 = features of samples 0..4095, rows 64:128 =
    # features of samples 4096..8191 (full-width DMA); w1 duplicated rows.
    obs_t = nc.dram_tensor("obs_t", [128, BC // 2], mdt, kind="ExternalInput").ap()
    relvel = nc.dram_tensor("relvel", [128, 8 * PC], F32, kind="ExternalInput").ap()
    w1t = nc.dram_tensor("w1t", [128, H], mdt, kind="ExternalInput").ap()
    w21t = nc.dram_tensor("w21t", [H, H], mdt, kind="ExternalInput").ap()
    w22t = nc.dram_tensor("w22t", [H, H], mdt, kind="ExternalInput").ap()
    wht = nc.dram_tensor("wht", [2 * H, 3], mdt, kind="ExternalInput").ap()
    b1p = nc.dram_tensor("b1p", [128, 2], F32, kind="ExternalInput").ap()
    b21p = nc.dram_tensor("b21p", [128, 2], F32, kind="ExternalInput").ap()
    b22p = nc.dram_tensor("b22p", [128, 2], F32, kind="ExternalInput").ap()
    bhp = nc.dram_tensor("bhp", [128, 1], F32, kind="ExternalInput").ap()
    u_out = nc.dram_tensor("u", [BC, 2], F32, kind="ExternalOutput").ap()

    with tile.TileContext(nc) as tc, ExitStack() as ctx:
        const = ctx.enter_context(tc.tile_pool(name="const", bufs=1))
        mpsum = ctx.enter_context(tc.tile_pool(name="mpsum", bufs=7, space="PSUM"))
        hpsum = ctx.enter_context(tc.tile_pool(name="hpsum", bufs=1, space="PSUM"))
        xpool = ctx.enter_context(tc.tile_pool(name="xp", bufs=3))
        ypool = ctx.enter_context(tc.tile_pool(name="yp", bufs=4))
        hsbp = ctx.enter_context(tc.tile_pool(name="hsb", bufs=2))
        ep = ctx.enter_context(tc.tile_pool(name="ep", bufs=2))
        plp = ctx.enter_context(tc.tile_pool(name="pl", bufs=2))

        # loads: latency-critical tensors on the SP HWDGE queue in first-
        # needed order; the rest via gpsimd SWDGE so the ACT engine is free
        # to start evictions immediately.
        w1_sb = const.tile([128, H], mdt, tag="w1", name="w1_sb")
        nc.sync.dma_start(out=w1_sb[:], in_=w1t[:])
        obs_sb = const.tile([128, BC // 2], mdt, tag="obs", name="obs_sb")
        nc.sync.dma_start(out=obs_sb[:, 0:1024], in_=obs_t[:, 0:1024])
        w21_sb = [const.tile([128, H], mdt, tag=f"w21_{k}", name=f"w21_sb{k}")
                  for k in range(2)]
        w22_sb = [const.tile([128, H], mdt, tag=f"w22_{k}", name=f"w22_sb{k}")
                  for k in range(2)]
        for k in range(2):
            nc.sync.dma_start(out=w21_sb[k][:], in_=w21t[k * 128:(k + 1) * 128, :])
        for q in range(1, 4):
            nc.sync.dma_start(out=obs_sb[:, q * 1024:(q + 1) * 1024],
                              in_=obs_t[:, q * 1024:(q + 1) * 1024])
        for k in range(2):
            nc.gpsimd.dma_start(out=w22_sb[k][:], in_=w22t[k * 128:(k + 1) * 128, :])
        wh_sb = [const.tile([128, 3], mdt, tag=f"wh_{k}", name=f"wh_sb{k}")
                 for k in range(4)]
        for k in range(4):
            nc.gpsimd.dma_start(out=wh_sb[k][:], in_=wht[k * 128:(k + 1) * 128, :])
        bh_sb = const.tile([128, 1], F32, tag="bh", name="bh_sb")
        nc.gpsimd.dma_start(out=bh_sb[:], in_=bhp[:])
        rv_sb = const.tile([128, 8 * PC], F32, tag="rv", name="rv_sb")
        nc.gpsimd.dma_start(out=rv_sb[:], in_=relvel[:])
        b1_sb = const.tile([128, 2], F32, tag="b1", name="b1_sb")
        b21_sb = const.tile([128, 2], F32, tag="b21", name="b21_sb")
        b22_sb = const.tile([128, 2], F32, tag="b22", name="b22_sb")
        if not zb:
            nc.gpsimd.dma_start(out=b1_sb[:], in_=b1p[:])
            nc.gpsimd.dma_start(out=b21_sb[:], in_=b21p[:])
            nc.gpsimd.dma_start(out=b22_sb[:], in_=b22p[:])
        eps8 = const.tile([128, 1], F32, tag="eps8", name="eps8")
        nc.vector.memset(eps8[:], 1e-8)

        # ---- PE warm-up + ACT table pin, hidden under the input DMAs ----
        scratch = const.tile([128, NT], mdt, tag="scr", name="scratch")
        nc.vector.memset(scratch[:], 0.0)
        scr2 = const.tile([128, 8], F32, tag="scr2", name="scratch2")
        nc.vector.memset(scr2[:], 1.0)
        nc.scalar.activation(scr2[:], scr2[:], AF.Ln)
        nc.scalar.activation(scr2[:], scr2[:], AF.Exp)
        wps = mpsum.tile([128, NT], F32, tag="ps", name="warm_ps")
        for _ in range(8):
            nc.tensor.matmul(wps[:], lhsT=mc(scratch[:, 0:128]),
                             rhs=mc(scratch[:]), start=True, stop=True)

        # ---- eviction engine alternation ----
        ev_state = [0]
        EV_PATTERN = "ADADA"  # 3 ACT : 2 DVE

        def evict(dst, src, bias_ap, relu=True):
            e = EV_PATTERN[ev_state[0] % len(EV_PATTERN)]
            ev_state[0] += 1
            if e == "A":
                if bias_ap is None:
                    nc.scalar.activation(dst, src, AF.Relu if relu else AF.Copy)
                else:
                    nc.scalar.activation(dst, src,
                                         AF.Relu if relu else AF.Identity,
                                         bias=bias_ap)
            else:
                if bias_ap is None:
                    if relu:
                        nc.vector.tensor_scalar(dst, src, 0.0, None, MAX)
                    else:
                        nc.vector.tensor_copy(dst, src)
                elif relu:
                    nc.vector.tensor_scalar(dst, src, bias_ap, 0.0, ADD, MAX)
                else:
                    nc.vector.tensor_scalar(dst, src, bias_ap, None, ADD)

        # ---- per-half head-output planes (written by spread DMAs) ----
        planes = []
        for h in range(2):
            planes.append({nm: plp.tile([128, PC], F32, tag=f"pl_{nm}",
                                        name=f"pl_{nm}_{h}")
                           for nm in ("ux", "uy", "bl")})

        V, S, G = nc.vector, nc.scalar, nc.gpsimd

        def T(tag):
            return ep.tile([128, PC], F32, tag=tag, name=f"ep_{tag}")[:]

        # ---- rel/vel-only subgraph: precomputed early (hidden in load) ----
        pre = [{}, {}]

        def pre_epilogue(h):
            rx = rv_sb[:, (0 + h) * PC:(1 + h) * PC]
            ry = rv_sb[:, (2 + h) * PC:(3 + h) * PC]
            vx = rv_sb[:, (4 + h) * PC:(5 + h) * PC]
            vy = rv_sb[:, (6 + h) * PC:(7 + h) * PC]
            t1 = T("t1"); V.tensor_tensor(t1, rx, rx, MULT)
            t2 = T("t2"); V.tensor_tensor(t2, ry, ry, MULT)
            rel2 = T("rel2"); V.tensor_tensor(rel2, t1, t2, ADD)
            d1 = T("d1"); G.tensor_tensor(d1, vx, rx, MULT)
            d2 = T("d2"); G.tensor_tensor(d2, vy, ry, MULT)
            ds = T("ds"); G.tensor_tensor(ds, d1, d2, ADD)
            X = T("X"); V.tensor_scalar(X, ds, 2.0, ALPHA * R_SAFE * R_SAFE,
                                        MULT, ADD)
            base = T("base"); V.scalar_tensor_tensor(base, rel2, -ALPHA, X,
                                                     MULT, ADD)
            sgm = []
            for mi, sd in enumerate(GMM_STDS):
                lr = T(f"lr{mi}")
                S.activation(lr, rel2, AF.Ln, bias=eps8[:], scale=4.0 * sd * sd)
                sf = T(f"sf{mi}")
                # sigma_f*INV_SQRT_2PI = exp(0.5*ln(v) + ln(INV_SQRT_2PI))
                S.activation(sf, lr, AF.Exp, scale=0.5)
                sg = T(f"sg{mi}")
                V.tensor_scalar(sg, sf, INV_SQRT_2PI, None, MULT)
                sgm.append(sg)
            den = T("den"); V.tensor_scalar(den, rel2, 4.0, 1e-12, MULT, ADD)
            rden = T("rden"); V.reciprocal(rden, den)
            pre[h] = dict(rx=rx, ry=ry, base=base, sgm=sgm, rden=rden)

        pre_epilogue(0)
        pre_epilogue(1)

        # ---- per-half epilogue chain (heads-dependent part) ----
        def epilogue(h):
            C = G if h == 0 else V  # hidden half -> gpsimd, exposed -> DVE
            p = pre[h]
            rx, ry, base, sgm, rden = p["rx"], p["ry"], p["base"], p["sgm"], p["rden"]
            ux = planes[h]["ux"][:]
            uy = planes[h]["uy"][:]
            bl = planes[h]["bl"][:]

            esg = T("esg"); S.activation(esg, bl, AF.Exp, scale=-1.0)
            rbeta = T("rbeta")
            V.tensor_scalar(rbeta, esg, 1.0 / BETA_MAX, 1.0 / BETA_MAX, MULT, ADD)
            V.tensor_scalar(rbeta, rbeta, 1e6, None, MIN)
            se = T("se"); V.tensor_scalar(se, esg, 1.0, None, ADD)
            rse = T("rse"); V.reciprocal(rse, se)
            beta = T("beta"); V.tensor_scalar(beta, rse, BETA_MAX, 1e-6, MULT, MAX)
            xb = T("xb"); V.tensor_scalar(xb, beta, -2.0, 1.0, MULT, ADD)
            tb = T("tb"); C.tensor_tensor(tb, xb, xb, MULT)
            wn = T("wn"); S.activation(wn, tb, AF.Ln, bias=1.0, scale=-1.0)
            wa = T("wa"); V.tensor_scalar(wa, wn, -1.0, -2.5, MULT, ADD)
            lw = T("lw"); S.activation(lw, wn, AF.Ln, scale=-1.0)
            sqw = T("sqw"); S.activation(sqw, lw, AF.Exp, scale=0.5)
            wb = T("wb"); V.tensor_scalar(wb, sqw, 3.0, None, SUB)

            # lateral: u-dependent dot product (off the beta chain)
            a1 = T("a1"); G.tensor_tensor(a1, rx, ux, MULT)
            a2 = T("a2"); G.tensor_tensor(a2, ry, uy, MULT)
            asum = T("as"); G.tensor_tensor(asum, a1, a2, ADD)

            def poly(w, C, tag, eng):
                pa = T(tag + "0"); pb_ = T(tag + "1")
                eng.tensor_scalar(pa, w, float(C[0]), None, MULT)
                cur, nxt = pa, pb_
                for c in C[1:8]:
                    eng.scalar_tensor_tensor(nxt, cur, float(c), w, ADD, MULT)
                    cur, nxt = nxt, cur
                out = T(tag + "r")
                eng.tensor_scalar(out, cur, float(C[8]), None, ADD)
                return out

            pa = poly(wa, ERFINV_A, "pA", V)
            pb = poly(wb, ERFINV_B, "pB", V)
            msk = ep.tile([128, PC], mybir.dt.uint8, tag="msk", name="ep_msk")[:]
            V.tensor_scalar(msk, wn, -5.0, None, IS_GT)
            psel = T("psel"); V.select(psel, msk, pa, pb)

            tt = T("tt"); C.tensor_tensor(tt, psel, xb, MULT)
            t2e = T("t2e"); C.tensor_tensor(t2e, tt, tt, MULT)
            pdfe = T("pdfe"); S.activation(pdfe, t2e, AF.Exp, scale=-1.0)
            cfp = T("cfp"); C.tensor_tensor(cfp, pdfe, rbeta, MULT)

            rhs = []
            for mi in range(3):
                tm = T(f"tm{mi}"); C.tensor_tensor(tm, sgm[mi], cfp, MULT)
                rr = T(f"rr{mi}"); C.tensor_tensor(rr, tm, base, ADD)
                rhs.append(rr)
            m1 = T("m1"); V.tensor_tensor(m1, rhs[0], rhs[1], MAX)
            mm_ = T("mm"); V.tensor_tensor(mm_, m1, rhs[2], MAX)
            es = []
            for mi in range(3):
                dd = T(f"dd{mi}"); V.tensor_tensor(dd, rhs[mi], mm_, SUB)
                ee = T(f"ee{mi}"); S.activation(ee, dd, AF.Exp, scale=1.0 / TAU)
                es.append(ee)
            s1 = T("s1"); C.tensor_tensor(s1, es[0], es[1], ADD)
            s2 = T("s2"); C.tensor_tensor(s2, s1, es[2], ADD)
            lse = T("lse"); S.activation(lse, s2, AF.Ln)
            rwc = T("rwc"); V.scalar_tensor_tensor(rwc, lse, TAU, mm_, MULT, ADD)

            viol = T("vi"); V.scalar_tensor_tensor(viol, asum, -2.0, rwc, MULT, ADD)
            vr = T("vr"); V.tensor_scalar(vr, viol, 0.0, None, MAX)
            stp = T("stp"); C.tensor_tensor(stp, vr, rden, MULT)
            sx = T("sx"); C.tensor_tensor(sx, stp, rx, MULT)
            sy = T("sy"); G.tensor_tensor(sy, stp, ry, MULT)
            uxy = plp.tile([128, 2 * PC], F32, tag="uxy", name="uxy")
            uxv = uxy.rearrange("p (c j) -> p c j", j=2)
            V.scalar_tensor_tensor(uxv[:, :, 0], sx, 2.0, ux, MULT, ADD)
            V.scalar_tensor_tensor(uxv[:, :, 1], sy, 2.0, uy, MULT, ADD)

            dst = u_out[h * HALF:(h + 1) * HALF, :].rearrange(
                "(p c) j -> p (c j)", p=128)
            nc.sync.dma_start(out=dst, in_=uxy[:])

        # ---- main batch loop ----
        # merged PSUM layout: one [128, 1024] tile holds both m-chunks;
        # heads for tile n-1 are emitted during iteration n so the PE never
        # stalls on the y-eviction it needs.
        pend = []

        def emit_heads_pair(items):
            (n0, yt0), (n1, yt1) = items
            hq = hpsum.tile([64, NT], F32, tag="hp", name="hp")

            def rhs_of(yt_, k):
                nm = "y21" if k < 2 else "y22"
                kk = k % 2
                return mc(yt_[nm][:, kk * NT:(kk + 1) * NT])

            for k in range(4):
                for off, yt_ in ((0, yt0), (32, yt1)):
                    nc.tensor.matmul(hq[off:off + 3, :], lhsT=mc(wh_sb[k][:]),
                                     rhs=rhs_of(yt_, k),
                                     start=(k == 0), stop=(k == 3))
            p2 = n1 // 2
            h_ = n1 // (NTILES // 2)
            hs = hsbp.tile([64, NT], F32, tag="hs", name="hs")
            evict(hs[:], hq[:], bh_sb[0:64, 0:1], relu=False)
            r0 = (p2 % 4) * 32
            for r, nm in enumerate(("ux", "uy", "bl")):
                nc.sync.dma_start(out=planes[h_][nm][r0:r0 + 32, :],
                                  in_=hs[r:r + 33:32, :])
            if n1 == NTILES // 2 - 1:
                epilogue(0)

        for n in range(NTILES):
            rows = slice(0, 64) if n < 8 else slice(64, 128)
            cc = (n % 8) * NT
            cols = slice(cc, cc + NT)
            # L1: one single-bank psum per m-chunk, evicted independently
            xt = xpool.tile([128, 2 * NT], mdt, tag="x", name="xt")
            ps1 = []
            for m in range(2):
                ps = mpsum.tile([128, NT], F32, tag="ps", name="ps1")
                nc.tensor.matmul(ps[:],
                                 lhsT=mc(w1_sb[rows, m * 128:(m + 1) * 128]),
                                 rhs=mc(obs_sb[rows, cols]), start=True, stop=True)
                ps1.append(ps)
            for m in range(2):
                evict(xt[:, m * NT:(m + 1) * NT], ps1[m][:],
                      None if zb else b1_sb[:, m:m + 1])
            # L21 / L22 (k-outer so LDWs prefetch under the other m's MM)
            yt = {}
            for wsb, bsb, nm in ((w21_sb, b21_sb, "y21"),
                                 (w22_sb, b22_sb, "y22")):
                pss = [mpsum.tile([128, NT], F32, tag="ps", name="ps2")
                       for _ in range(2)]
                for k in range(2):
                    for m in range(2):
                        nc.tensor.matmul(
                            pss[m][:],
                            lhsT=mc(wsb[k][:, m * 128:(m + 1) * 128]),
                            rhs=mc(xt[:, k * NT:(k + 1) * NT]),
                            start=(k == 0), stop=(k == 1))
                t = ypool.tile([128, 2 * NT], mdt, tag=nm, name=nm)
                for m in range(2):
                    evict(t[:, m * NT:(m + 1) * NT], pss[m][:],
                          None if zb else bsb[:, m:m + 1])
                yt[nm] = t
            if len(pend) == 2:
                emit_heads_pair(pend)
                pend = []
            pend.append((n, yt))
        emit_heads_pair(pend)
        epilogue(1)

    _split_multi_waits(nc)
    return nc


_CACHE = {}


def _get_nc(mm, zb):
    key = (mm, zb)
    if key not in _CACHE:
        _CACHE[key] = _build(mm, zb)
    return _CACHE[key]


def _prep_inputs(inputs, mm):
    obs = np.ascontiguousarray(inputs["obs"], dtype=np.float32)
    W1 = np.asarray(inputs["W1"], np.float32)
    b1 = np.asarray(inputs["b1"], np.float32)
    W21 = np.asarray(inputs["W21"], np.float32)
    b21 = np.asarray(inputs["b21"], np.float32)
    W22 = np.asarray(inputs["W22"], np.float32)
    b22 = np.asarray(inputs["b22"], np.float32)
    W31 = np.asarray(inputs["W31"], np.float32)
    b31 = np.asarray(inputs["b31"], np.float32)
    W32 = np.asarray(inputs["W32"], np.float32)
    b32 = np.asarray(inputs["b32"], np.float32)

    npdt = _NPDT[mm]

    w1t = np.ascontiguousarray(np.vstack([W1.T, W1.T])).astype(npdt)
    w21t = np.ascontiguousarray(W21.T).astype(npdt)
    w22t = np.ascontiguousarray(W22.T).astype(npdt)
    wht = np.zeros((2 * H, 3), np.float32)
    wht[0:H, 0:2] = W31.T
    wht[H:2 * H, 2] = W32[0]
    wht = wht.astype(npdt)
    b1p = np.ascontiguousarray(b1.reshape(2, 128).T)
    b21p = np.ascontiguousarray(b21.reshape(2, 128).T)
    b22p = np.ascontiguousarray(b22.reshape(2, 128).T)
    bhp = np.zeros((128, 1), np.float32)   # pair-packed head bias pattern
    for j in range(4):
        bhp[32 * j + 0, 0] = b31[0]
        bhp[32 * j + 1, 0] = b31[1]
        bhp[32 * j + 2, 0] = b32[0]
    zb = not (b1.any() or b21.any() or b22.any())

    in_maps = []
    for c in range(N_CORES):
        oc = obs[c * BC:(c + 1) * BC]                          # [8192, 64]
        obs_tc = np.empty((128, BC // 2), np.float32)
        obs_tc[0:64] = oc[0:BC // 2].T
        obs_tc[64:128] = oc[BC // 2:].T
        obs_tc = np.ascontiguousarray(obs_tc).astype(npdt)     # [128, 4096]
        rv = np.empty((128, 8 * PC), np.float32)
        for v, col in enumerate((6, 7, 8, 9)):
            pl = oc[:, col].reshape(2, 128, PC)                # [h][128, 32]
            rv[:, (2 * v) * PC:(2 * v + 1) * PC] = pl[0]
            rv[:, (2 * v + 1) * PC:(2 * v + 2) * PC] = pl[1]
        in_maps.append({
            "obs_t": obs_tc, "relvel": rv,
            "w1t": w1t, "w21t": w21t, "w22t": w22t, "wht": wht,
            "b1p": b1p, "b21p": b21p, "b22p": b22p, "bhp": bhp,
        })
    return in_maps, zb


def _install_ntff_hook_shim():
    """antenv.axon_hooks is absent in this image; recreate it from the
    boot helper so run_bass_kernel_spmd(trace=True) can NTFF-profile."""
    import types
    try:
        import antenv.axon_hooks  # noqa: F401
        return
    except ImportError:
        pass
    import antenv
    from trn_agent_boot.trn_boot import _ntff_profile_via_ctypes
    mod = types.ModuleType("antenv.axon_hooks")
    state = {"hook": _ntff_profile_via_ctypes("/opt/axon/libaxon_pjrt.so")}
    mod.set_axon_ntff_profile_hook = lambda h: state.__setitem__("hook", h)
    mod.get_axon_ntff_profile_hook = lambda: state["hook"]
    sys.modules["antenv.axon_hooks"] = mod
    antenv.axon_hooks = mod


def run(inputs, mm=None, trace=False, trace_kwargs=None):
    mm = mm or os.environ.get("BARRIER_MM_DTYPE", "fp16")
    if trace:
        _install_ntff_hook_shim()
    in_maps, zb = _prep_inputs(inputs, mm)
    nc = _get_nc(mm, zb)
    res = run_bass_kernel_spmd(nc, in_maps, list(range(N_CORES)),
                               trace=trace, **(trace_kwargs or {}))
    out = np.concatenate([res.results[c]["u"] for c in range(N_CORES)], axis=0)
    return out.astype(np.float32), res


def kernel(**inputs):
    out, _ = run(inputs)
    return out
